# revision 4
# baseline (speedup 1.0000x reference)
"""Trainium2 Bass kernel for nn_CausalSelfAttention (B=2, T=4096, D=512, H=8, hd=64).

Sharding: batch x head-pair over 8 cores (core i: batch i//4, heads 2*(i%4), 2*(i%4)+1).
Each core computes QKV projection + RoPE + full-T causal attention for its 2 heads and
a partial output projection (row-parallel c_proj); host sums the 4 partials per batch.

v2 design (vs 279us baseline): the baseline was PE-bound but ran the PE at
1.2 GHz most of the span (HAM throttle 75%) because the PE and ACT streams
serialized in a ping-pong. This version rebalances the engines and keeps the
PE stream dense:
  - ACT does exp ONLY (was: exp + 28us of copies). Exp in groups of 4
    128-k-chunks from fp16 PSUM tiles [128,2048] (2 banks) -> (2048+352)/1.2
    = 2us per group, Jb+1 groups per (q-block, head): ~144us total.
  - PE sheds ~37K cycles: the rotate-half projections (wqb/wkb) are replaced
    by one [128x128] permutation matmul each (sign folded into host-prepped
    ss); V^T is computed directly in transposed form from xT chunks (kills
    the per-chunk PE transposes).
  - Projections for j-block Jb+2 and the y-projection of Jb-1 are emitted as
    filler pieces BETWEEN attention S/AV groups of block Jb so the in-order
    PE queue never sits idle long enough for the HAM clock gate (>3.4us) to
    re-throttle.
  - Normalization: 1/l via reciprocal_approx_fast straight from the PSUM
    ones-row, gpsimd partition_broadcast, one fused TT-mul writing fp16.
  - y projection into one fp16 PSUM tile [128, 4*512] (2 banks), single DVE
    copy, fp16 DMA out; host divides nothing (o already normalized), just
    transposes and accumulates the 4 head-pair partials per batch.
PSUM budget: s 2x2 + o 1 + y 2 + proj 1 = 8 banks.
"""

import sys

sys.path.insert(0, "/opt/trn_rl_repo")

from collections import deque
from contextlib import ExitStack

import ml_dtypes
import numpy as np

import concourse.bass as bass
import concourse.tile as tile
from concourse import bacc, mybir
from concourse.bass import ts
from concourse.bass_utils import run_bass_kernel_spmd

F32 = mybir.dt.float32
F16 = mybir.dt.float16

B, C, H, HD = 2, 512, 8, 64
N_CORES = 8


def build_kernel(T=4096, n_cores=N_CORES):
    nc = bacc.Bacc(
        "TRN2",
        target_bir_lowering=False,
        debug=False,
        num_devices=n_cores,
    )
    NJ = T // 512
    NK = T // 128
    QB = 512
    NB = T // QB
    LAG = 2

    xT_d = nc.dram_tensor("xT", [C, T], F16, kind="ExternalInput").ap()
    cc_d = nc.dram_tensor("ccT", [128, T], F16, kind="ExternalInput").ap()
    ss_d = nc.dram_tensor("ssT", [128, T], F16, kind="ExternalInput").ap()
    w_d = {}
    for name in ("wqT", "wkT", "wvT"):
        w_d[name] = nc.dram_tensor(name, [C, 128], F16, kind="ExternalInput").ap()
    wp_d = nc.dram_tensor("wpT", [128, C], F16, kind="ExternalInput").ap()
    msk_d = nc.dram_tensor("masks", [128, 4, QB], F16, kind="ExternalInput").ap()
    perm_d = nc.dram_tensor("perm", [128, 128], F16, kind="ExternalInput").ap()
    y_d = nc.dram_tensor("yT", [C, T], F16, kind="ExternalOutput").ap()
    warm_d = nc.dram_tensor("warm", [1, 4], F32, kind="ExternalOutput").ap()

    SCALE = float(1.0 / np.sqrt(HD))

    with tile.TileContext(nc) as tc, ExitStack() as ctx:
        consts = ctx.enter_context(tc.tile_pool(name="consts", bufs=1))
        big = ctx.enter_context(tc.tile_pool(name="big", bufs=1))
        xpool = ctx.enter_context(tc.tile_pool(name="xpool", bufs=12))
        qpool = ctx.enter_context(tc.tile_pool(name="qpool", bufs=4))
        rpool = ctx.enter_context(tc.tile_pool(name="rpool", bufs=6))
        epool = ctx.enter_context(tc.tile_pool(name="epool", bufs=4))
        opool = ctx.enter_context(tc.tile_pool(name="opool", bufs=3))
        spool = ctx.enter_context(tc.tile_pool(name="small", bufs=4))
        ypool = ctx.enter_context(tc.tile_pool(name="ypool", bufs=2))

        w_sb = {}
        for name in ("wqT", "wkT", "wvT"):
            w = consts.tile([128, 4, 128], F16, tag=name, name=f"w_{name}")
            nc.sync.dma_start(w[:], w_d[name].rearrange("(c p) m -> p c m", c=4))
            w_sb[name] = w
        perm = consts.tile([128, 128], F16)
        nc.sync.dma_start(perm[:], perm_d[:])

        krT = big.tile([128, T], F16)
        v_aug = big.tile([128, 2, NK, 65], F16)
        nc.gpsimd.memset(v_aug[:], 1.0)

        ps_aux = ctx.enter_context(tc.tile_pool(name="ps_aux", bufs=1, space="PSUM"))
        ps_s = ctx.enter_context(tc.tile_pool(name="ps_s", bufs=3, space="PSUM"))
        ps_o = ctx.enter_context(tc.tile_pool(name="ps_o", bufs=1, space="PSUM"))

        # ---- PE warmup burst: release the HAM clock gate before real work.
        wz = xpool.tile([128, 512], F16, tag="xc")
        nc.gpsimd.memset(wz[:], 0.25)
        wu_ps = ps_aux.tile([128, 512], F32, tag="p")
        for _ in range(10):
            nc.tensor.matmul(wu_ps[:], wz[:, 0:128], wz[:], start=True, stop=True)
        wexp = spool.tile([1, 4], F16, tag="wexp")
        nc.scalar.activation(wexp[:], wu_ps[0:1, 0:4],
                             mybir.ActivationFunctionType.Exp, scale=0.001)
        wsink = spool.tile([1, 4], F32, tag="wsink")
        nc.vector.tensor_copy(wsink[:], wu_ps[0:1, 0:4])
        nc.sync.dma_start(warm_d[:], wsink[:])

        # late-loaded consts (keep first x/w DMAs alone on the critical path)
        cc = consts.tile([128, T], F16, name="cc")
        ss = consts.tile([128, T], F16, name="ss")
        masks = consts.tile([128, 4, QB], F16, name="masks")
        w_p = consts.tile([128, C], F16, name="wp")
        _late = [False]

        qr_tiles = {}
        o_tiles = {}

        def proj_pieces(j):
            """Emit-able pieces of the j-th projection block. Each piece is a
            short PE burst; DVE/DMA consumers run while later pieces and the
            surrounding attention groups keep the PE busy."""
            jc = ts(j, 512)
            st = {}

            def p_x():
                xc = []
                for c in range(4):
                    xt = xpool.tile([128, 512], F16, tag="xc")
                    nc.sync.dma_start(xt[:], xT_d[ts(c, 128), jc])
                    xc.append(xt)
                st["xc"] = xc
                if not _late[0]:
                    _late[0] = True
                    nc.gpsimd.dma_start(cc[:], cc_d[:])
                    nc.gpsimd.dma_start(ss[:], ss_d[:])
                    nc.sync.dma_start(masks[:], msk_d[:])
                    nc.sync.dma_start(w_p[:], wp_d[:])

            def mk_qk(name, out_tag):
                def piece():
                    ps = ps_aux.tile([128, 512], F32, tag="p", name=f"ps_{name}_{j}")
                    for c in range(4):
                        nc.tensor.matmul(
                            ps[:], w_sb[name][:, c, :], st["xc"][c][:],
                            start=(c == 0), stop=(c == 3),
                        )
                    a_sb = qpool.tile([128, 512], F16, tag="a")
                    nc.vector.tensor_copy(a_sb[:], ps[:])
                    st[out_tag] = a_sb
                return piece

            def mk_rope(a_tag, out_name):
                def piece():
                    # qb = perm.T @ qa (the rotate-half partition swap on PE)
                    b_ps = ps_aux.tile([128, 512], F32, tag="p", name=f"ps_b_{out_name}_{j}")
                    nc.tensor.matmul(b_ps[:], perm[:], st[a_tag][:], start=True, stop=True)
                    b_sb = qpool.tile([128, 512], F16, tag="b")
                    nc.vector.tensor_copy(b_sb[:], b_ps[:])
                    m1 = rpool.tile([128, 512], F16, tag="m1")
                    m2 = rpool.tile([128, 512], F16, tag="m2")
                    nc.vector.tensor_mul(m1[:], st[a_tag][:], cc[:, jc])
                    nc.vector.tensor_mul(m2[:], b_sb[:], ss[:, jc])
                    if out_name == "q":
                        qr = qpool.tile([128, 512], F16, tag="qr", name=f"qr_{j}")
                        nc.vector.tensor_add(qr[:], m1[:], m2[:])
                        qr_tiles[j] = qr
                    else:
                        nc.vector.tensor_add(krT[:, jc], m1[:], m2[:])
                return piece

            def mk_vt(kc):
                def piece():
                    vt_ps = ps_aux.tile([128, 128], F32, tag="p", name=f"ps_vt_{j}_{kc}")
                    for c in range(4):
                        nc.tensor.matmul(
                            vt_ps[:], st["xc"][c][:, ts(kc, 128)], w_sb["wvT"][:, c, :],
                            start=(c == 0), stop=(c == 3),
                        )
                    cci = 4 * j + kc
                    nc.vector.tensor_copy(
                        v_aug[:, :, cci, 0:64],
                        vt_ps[:].rearrange("p (h d) -> p h d", h=2),
                    )
                return piece

            return [
                p_x,
                mk_qk("wqT", "qa"),
                mk_rope("qa", "q"),
                mk_qk("wkT", "ka"),
                mk_rope("ka", "k"),
                mk_vt(0), mk_vt(1), mk_vt(2), mk_vt(3),
            ]

        def y_pieces(Jb):
            jc = ts(Jb, QB)

            def mk(c):
                def piece():
                    oT = o_tiles[Jb]
                    if c == 3:
                        o_tiles.pop(Jb)
                    y_ps = ps_aux.tile([128, QB], F32, tag="p", name=f"ps_y_{Jb}_{c}")
                    nc.tensor.matmul(
                        y_ps[:], w_p[:, ts(c, 128)], oT[:], start=True, stop=True
                    )
                    y_sb = ypool.tile([128, QB], F16, tag="ysb")
                    nc.vector.tensor_copy(y_sb[:], y_ps[:])
                    nc.sync.dma_start(y_d[ts(c, 128), jc], y_sb[:])
                return piece

            return [mk(0), mk(1), mk(2), mk(3)]

        fillers = deque()
        # prologue: projections for j=0,1 emitted straight
        for piece in proj_pieces(0):
            piece()
        for piece in proj_pieces(1):
            piece()

        for Jb in range(NB):
            jc = ts(Jb, QB)
            if Jb + 2 < NJ:
                fillers.extend(proj_pieces(Jb + 2))
            for h in range(2):
                r = 64 * h
                nchunks = 4 * (Jb + 1)
                ngroups = nchunks // 2
                o_ps = ps_o.tile([65, QB], F32, tag="o", name=f"ps_o_{Jb}_{h}")
                e_tiles = {}
                qr = qr_tiles[Jb]

                def emit_av(g):
                    e_sb = e_tiles.pop(g)
                    for ci in range(2):
                        cci = 2 * g + ci
                        nc.tensor.matmul(
                            o_ps[:],
                            v_aug[:, h, cci, :],
                            e_sb[:, ts(ci, QB)],
                            start=(cci == 0),
                            stop=(cci == nchunks - 1),
                        )

                for g in range(ngroups + LAG):
                    if g < ngroups:
                        s_ps = ps_s.tile([128, 2 * QB], F32, tag="s", name=f"ps_s_{Jb}_{h}_{g}")
                        for ci in range(2):
                            cci = 2 * g + ci
                            nc.tensor.matmul(
                                s_ps[:, ts(ci, QB)],
                                krT[r : r + 64, ts(cci, 128)],
                                qr[r : r + 64, :],
                                start=True,
                                stop=True,
                            )
                        e_sb = epool.tile([128, 2 * QB], F16, tag="e")
                        nc.scalar.activation(
                            e_sb[:], s_ps[:], mybir.ActivationFunctionType.Exp,
                            scale=SCALE,
                        )
                        if g >= ngroups - 2:
                            for ci in range(2):
                                m = 2 * g + ci - 4 * Jb
                                nc.vector.tensor_mul(
                                    e_sb[:, ts(ci, QB)], e_sb[:, ts(ci, QB)],
                                    masks[:, m, :],
                                )
                        e_tiles[g] = e_sb
                    if g >= LAG:
                        emit_av(g - LAG)
                    if fillers:
                        fillers.popleft()()

                # normalize: oT[h] = o * (1/l); recip straight off the PSUM
                # ones-row, broadcast on gpsimd, one fused mul writing fp16
                if h == 0:
                    oT = opool.tile([128, QB], F16, tag="oT", name=f"oT_{Jb}")
                    o_tiles[Jb] = oT
                else:
                    oT = o_tiles[Jb]
                l_sb = spool.tile([1, QB], F32, tag="lsb")
                nc.vector.tensor_copy(l_sb[:], o_ps[64:65, :])
                rb = spool.tile([1, QB], F32, tag="rb")
                nc.vector.reciprocal_approx_fast(rb[:], l_sb[:])
                bc = spool.tile([64, QB], F32, tag="bc")
                nc.gpsimd.partition_broadcast(bc[:], rb[:])
                nc.vector.tensor_mul(oT[r : r + 64, :], o_ps[0:64, :], bc[:])

            fillers.extend(y_pieces(Jb))

        while fillers:
            fillers.popleft()()

    nc.compile()
    return nc


# ---------------- host-side wrapper ----------------

_CACHE = {}


def _get_nc(T):
    if T not in _CACHE:
        _CACHE[T] = build_kernel(T)
    return _CACHE[T]


def _host_prep(x, cos, sin, Wq, Wk, Wv, Wp):
    T = x.shape[1]
    cosT = np.ascontiguousarray(cos.T).astype(np.float32)  # [32, T]
    sinT = np.ascontiguousarray(sin.T).astype(np.float32)
    ccT = np.concatenate([cosT] * 4, axis=0).astype(np.float16)  # [128, T]
    sgn = np.where((np.arange(128) % 64) < 32, 1.0, -1.0)[:, None].astype(np.float32)
    ssT = (np.concatenate([sinT] * 4, axis=0) * sgn).astype(np.float16)
    rr = np.arange(128)[:, None]
    cq = np.arange(512)[None, :]
    masks = np.stack(
        [(cq >= 128 * m + rr) for m in range(4)], axis=1
    ).astype(np.float16)  # [128, 4, 512]
    # qb = perm.T @ qa: qb[d] = qa[swap(d)], swap = +-32 within each 64-row head
    dd = np.arange(128)
    swap = np.where((dd % 64) < 32, dd + 32, dd - 32)
    permM = np.zeros((128, 128), np.float16)
    permM[swap, dd] = 1.0

    in_maps = []
    for core in range(N_CORES):
        b, p = core // 4, core % 4
        hs = slice(128 * p, 128 * (p + 1))
        in_maps.append(
            {
                "xT": np.ascontiguousarray(x[b].T.astype(np.float16)),
                "ccT": ccT,
                "ssT": ssT,
                "wqT": np.ascontiguousarray(Wq[hs].T).astype(np.float16),
                "wkT": np.ascontiguousarray(Wk[hs].T).astype(np.float16),
                "wvT": np.ascontiguousarray(Wv[hs].T).astype(np.float16),
                "wpT": np.ascontiguousarray(Wp[:, hs].T.astype(np.float16)),
                "masks": masks,
                "perm": permM,
            }
        )
    return in_maps


def kernel(x, cos, sin, Wq, Wk, Wv, Wp, _trace=False, _nc=None):
    x = np.asarray(x)
    T = x.shape[1]
    nc = _nc if _nc is not None else _get_nc(T)
    in_maps = _host_prep(
        x, np.asarray(cos), np.asarray(sin),
        np.asarray(Wq), np.asarray(Wk), np.asarray(Wv), np.asarray(Wp),
    )
    res = run_bass_kernel_spmd(nc, in_maps, list(range(N_CORES)), trace=_trace)
    y = np.zeros((B, T, C), np.float32)
    for core in range(N_CORES):
        y[core // 4] += res.results[core]["yT"].T.astype(np.float32)
    kernel.last_results = res
    return y


# revision 6
# speedup vs baseline: 1.0975x; 1.0975x over previous
"""Trainium2 Bass kernel for nn_CausalSelfAttention (B=2, T=4096, D=512, H=8, hd=64).

Sharding: batch x head-pair over 8 cores (core i: batch i//4, heads 2*(i%4), 2*(i%4)+1).
Each core computes QKV projection + RoPE + full-T causal attention for its 2 heads and
a partial output projection (row-parallel c_proj); host sums the 4 partials per batch.

v2 design (vs 279us baseline): the baseline was PE-bound but ran the PE at
1.2 GHz most of the span (HAM throttle 75%) because the PE and ACT streams
serialized in a ping-pong. This version rebalances the engines and keeps the
PE stream dense:
  - ACT does exp ONLY (was: exp + 28us of copies). Exp in groups of 4
    128-k-chunks from fp16 PSUM tiles [128,2048] (2 banks) -> (2048+352)/1.2
    = 2us per group, Jb+1 groups per (q-block, head): ~144us total.
  - PE sheds ~37K cycles: the rotate-half projections (wqb/wkb) are replaced
    by one [128x128] permutation matmul each (sign folded into host-prepped
    ss); V^T is computed directly in transposed form from xT chunks (kills
    the per-chunk PE transposes).
  - Projections for j-block Jb+2 and the y-projection of Jb-1 are emitted as
    filler pieces BETWEEN attention S/AV groups of block Jb so the in-order
    PE queue never sits idle long enough for the HAM clock gate (>3.4us) to
    re-throttle.
  - Normalization: 1/l via reciprocal_approx_fast straight from the PSUM
    ones-row, gpsimd partition_broadcast, one fused TT-mul writing fp16.
  - y projection into one fp16 PSUM tile [128, 4*512] (2 banks), single DVE
    copy, fp16 DMA out; host divides nothing (o already normalized), just
    transposes and accumulates the 4 head-pair partials per batch.
PSUM budget: s 2x2 + o 1 + y 2 + proj 1 = 8 banks.
"""

import sys

sys.path.insert(0, "/opt/trn_rl_repo")

from collections import deque
from contextlib import ExitStack

import ml_dtypes
import numpy as np

import concourse.bass as bass
import concourse.tile as tile
from concourse import bacc, mybir
from concourse.bass import ts
from concourse.bass_utils import run_bass_kernel_spmd

F32 = mybir.dt.float32
F16 = mybir.dt.float16

B, C, H, HD = 2, 512, 8, 64
N_CORES = 8


def build_kernel(T=4096, n_cores=N_CORES):
    nc = bacc.Bacc(
        "TRN2",
        target_bir_lowering=False,
        debug=False,
        num_devices=n_cores,
    )
    NJ = T // 512
    NK = T // 128
    QB = 512
    NB = T // QB
    LAG = 2
    GS = 3

    xT_d = nc.dram_tensor("xT", [C, T], F16, kind="ExternalInput").ap()
    cc_d = nc.dram_tensor("ccT", [128, T], F16, kind="ExternalInput").ap()
    ss_d = nc.dram_tensor("ssT", [128, T], F16, kind="ExternalInput").ap()
    w_d = {}
    for name in ("wqT", "wkT", "wvT"):
        w_d[name] = nc.dram_tensor(name, [C, 128], F16, kind="ExternalInput").ap()
    wp_d = nc.dram_tensor("wpT", [128, C], F16, kind="ExternalInput").ap()
    msk_d = nc.dram_tensor("masks", [128, 4, QB], F16, kind="ExternalInput").ap()
    perm_d = nc.dram_tensor("perm", [128, 128], F16, kind="ExternalInput").ap()
    y_d = nc.dram_tensor("yT", [C, T], F16, kind="ExternalOutput").ap()
    warm_d = nc.dram_tensor("warm", [1, 4], F32, kind="ExternalOutput").ap()

    SCALE = float(1.0 / np.sqrt(HD))

    with tile.TileContext(nc) as tc, ExitStack() as ctx:
        consts = ctx.enter_context(tc.tile_pool(name="consts", bufs=1))
        big = ctx.enter_context(tc.tile_pool(name="big", bufs=1))
        xpool = ctx.enter_context(tc.tile_pool(name="xpool", bufs=12))
        qpool = ctx.enter_context(tc.tile_pool(name="qpool", bufs=4))
        rpool = ctx.enter_context(tc.tile_pool(name="rpool", bufs=6))
        epool = ctx.enter_context(tc.tile_pool(name="epool", bufs=5))
        opool = ctx.enter_context(tc.tile_pool(name="opool", bufs=3))
        spool = ctx.enter_context(tc.tile_pool(name="small", bufs=4))
        ypool = ctx.enter_context(tc.tile_pool(name="ypool", bufs=2))

        w_sb = {}
        for name in ("wqT", "wkT", "wvT"):
            w = consts.tile([128, 4, 128], F16, tag=name, name=f"w_{name}")
            nc.sync.dma_start(w[:], w_d[name].rearrange("(c p) m -> p c m", c=4))
            w_sb[name] = w
        perm = consts.tile([128, 128], F16)
        nc.sync.dma_start(perm[:], perm_d[:])

        krT = big.tile([128, T], F16)
        v_aug = big.tile([128, 2, NK, 65], F16)
        nc.gpsimd.memset(v_aug[:], 1.0)

        ps_aux = ctx.enter_context(tc.tile_pool(name="ps_aux", bufs=1, space="PSUM"))
        ps_s = ctx.enter_context(tc.tile_pool(name="ps_s", bufs=2, space="PSUM"))
        ps_o = ctx.enter_context(tc.tile_pool(name="ps_o", bufs=1, space="PSUM"))

        # ---- PE warmup burst: release the HAM clock gate before real work.
        wz = xpool.tile([128, 512], F16, tag="xc")
        nc.gpsimd.memset(wz[:], 0.25)
        wu_ps = ps_aux.tile([128, 512], F32, tag="p")
        for _ in range(10):
            nc.tensor.matmul(wu_ps[:], wz[:, 0:128], wz[:], start=True, stop=True)
        wexp = spool.tile([1, 4], F16, tag="wexp")
        nc.scalar.activation(wexp[:], wu_ps[0:1, 0:4],
                             mybir.ActivationFunctionType.Exp, scale=0.001)
        wsink = spool.tile([1, 4], F32, tag="wsink")
        nc.vector.tensor_copy(wsink[:], wu_ps[0:1, 0:4])
        nc.sync.dma_start(warm_d[:], wsink[:])

        # late-loaded consts (keep first x/w DMAs alone on the critical path)
        cc = consts.tile([128, T], F16, name="cc")
        ss = consts.tile([128, T], F16, name="ss")
        masks = consts.tile([128, 4, QB], F16, name="masks")
        w_p = consts.tile([128, C], F16, name="wp")
        _late = [False]

        qr_tiles = {}
        o_tiles = {}

        def proj_pieces(j, pool=None, tag="p"):
            """Emit-able pieces of the j-th projection block. Each piece is a
            short PE burst; DVE/DMA consumers run while later pieces and the
            surrounding attention groups keep the PE busy."""
            jc = ts(j, 512)
            st = {"pool": pool if pool is not None else ps_aux, "tag": tag}

            def p_x():
                xc = []
                for c in range(4):
                    xt = xpool.tile([128, 512], F16, tag="xc")
                    nc.sync.dma_start(xt[:], xT_d[ts(c, 128), jc])
                    xc.append(xt)
                st["xc"] = xc
                if not _late[0]:
                    _late[0] = True
                    nc.gpsimd.dma_start(cc[:], cc_d[:])
                    nc.gpsimd.dma_start(ss[:], ss_d[:])
                    nc.sync.dma_start(masks[:], msk_d[:])
                    nc.sync.dma_start(w_p[:], wp_d[:])

            def mk_qk(name, out_tag):
                def piece():
                    ps = st["pool"].tile([128, 512], F32, tag=st["tag"], name=f"ps_{name}_{j}")
                    for c in range(4):
                        nc.tensor.matmul(
                            ps[:], w_sb[name][:, c, :], st["xc"][c][:],
                            start=(c == 0), stop=(c == 3),
                        )
                    a_sb = qpool.tile([128, 512], F16, tag="a")
                    nc.vector.tensor_copy(a_sb[:], ps[:])
                    st[out_tag] = a_sb
                return piece

            def mk_rope(a_tag, out_name):
                def piece():
                    # qb = perm.T @ qa (the rotate-half partition swap on PE)
                    b_ps = st["pool"].tile([128, 512], F32, tag=st["tag"], name=f"ps_b_{out_name}_{j}")
                    nc.tensor.matmul(b_ps[:], perm[:], st[a_tag][:], start=True, stop=True)
                    m1 = rpool.tile([128, 512], F16, tag="m1")
                    m2 = rpool.tile([128, 512], F16, tag="m2")
                    nc.vector.tensor_mul(m1[:], st[a_tag][:], cc[:, jc])
                    nc.vector.tensor_mul(m2[:], b_ps[:], ss[:, jc])
                    if out_name == "q":
                        qr = qpool.tile([128, 512], F16, tag="qr", name=f"qr_{j}")
                        nc.vector.tensor_add(qr[:], m1[:], m2[:])
                        qr_tiles[j] = qr
                    else:
                        nc.vector.tensor_add(krT[:, jc], m1[:], m2[:])
                return piece

            def p_vt():
                vt_ps = st["pool"].tile([128, 512], F32, tag=st["tag"], name=f"ps_vt_{j}")
                for kc in range(4):
                    for c in range(4):
                        nc.tensor.matmul(
                            vt_ps[:, ts(kc, 128)],
                            st["xc"][c][:, ts(kc, 128)], w_sb["wvT"][:, c, :],
                            start=(c == 0), stop=(c == 3),
                        )
                vw = vt_ps[:].rearrange("p (kc h d) -> p kc h d", kc=4, h=2)
                for hh in range(2):
                    nc.vector.tensor_copy(
                        v_aug[:, hh, 4 * j : 4 * j + 4, 0:64], vw[:, :, hh, :]
                    )

            return [
                p_x,
                mk_qk("wqT", "qa"),
                mk_rope("qa", "q"),
                mk_qk("wkT", "ka"),
                mk_rope("ka", "k"),
                p_vt,
            ]

        def y_pieces(Jb):
            jc = ts(Jb, QB)

            def mk(c):
                def piece():
                    oT = o_tiles[Jb]
                    if c == 3:
                        o_tiles.pop(Jb)
                    y_ps = ps_aux.tile([128, QB], F32, tag="p", name=f"ps_y_{Jb}_{c}")
                    nc.tensor.matmul(
                        y_ps[:], w_p[:, ts(c, 128)], oT[:], start=True, stop=True
                    )
                    y_sb = ypool.tile([128, QB], F16, tag="ysb")
                    nc.vector.tensor_copy(y_sb[:], y_ps[:])
                    nc.sync.dma_start(y_d[ts(c, 128), jc], y_sb[:])
                return piece

            return [mk(0), mk(1), mk(2), mk(3)]

        fillers = deque()
        # prologue: projections for j=0,1 interleaved through the two ps_s bufs
        pp0 = proj_pieces(0, pool=ps_s, tag="s")
        pp1 = proj_pieces(1, pool=ps_s, tag="s")
        for a, b in zip(pp0, pp1):
            a()
            b()

        for Jb in range(NB):
            jc = ts(Jb, QB)
            if Jb + 2 < NJ:
                fillers.extend(proj_pieces(Jb + 2))
            for h in range(2):
                r = 64 * h
                nchunks = 4 * (Jb + 1)
                ngroups = (nchunks + GS - 1) // GS
                o_ps = ps_o.tile([65, QB], F32, tag="o", name=f"ps_o_{Jb}_{h}")
                e_tiles = {}
                qr = qr_tiles[Jb]

                def emit_av(g):
                    e_sb = e_tiles.pop(g)
                    for ci in range(min(GS, nchunks - GS * g)):
                        cci = GS * g + ci
                        nc.tensor.matmul(
                            o_ps[:],
                            v_aug[:, h, cci, :],
                            e_sb[:, ts(ci, QB)],
                            start=(cci == 0),
                            stop=(cci == nchunks - 1),
                        )

                for g in range(ngroups + LAG):
                    if g < ngroups:
                        gn = min(GS, nchunks - GS * g)
                        s_ps = ps_s.tile([128, GS * QB], F32, tag="s", name=f"ps_s_{Jb}_{h}_{g}")
                        for ci in range(gn):
                            cci = GS * g + ci
                            nc.tensor.matmul(
                                s_ps[:, ts(ci, QB)],
                                krT[r : r + 64, ts(cci, 128)],
                                qr[r : r + 64, :],
                                start=True,
                                stop=True,
                            )
                        e_sb = epool.tile([128, GS * QB], F16, tag="e")
                        nc.scalar.activation(
                            e_sb[:, 0 : gn * QB], s_ps[:, 0 : gn * QB],
                            mybir.ActivationFunctionType.Exp,
                            scale=SCALE,
                        )
                        for ci in range(gn):
                            m = GS * g + ci - 4 * Jb
                            if m >= 0:
                                nc.vector.tensor_mul(
                                    e_sb[:, ts(ci, QB)], e_sb[:, ts(ci, QB)],
                                    masks[:, m, :],
                                )
                        e_tiles[g] = e_sb
                    if g >= LAG:
                        emit_av(g - LAG)
                    if fillers:
                        fillers.popleft()()

                # normalize: oT[h] = o * (1/l); recip straight off the PSUM
                # ones-row, broadcast on gpsimd, one fused mul writing fp16
                if h == 0:
                    oT = opool.tile([128, QB], F16, tag="oT", name=f"oT_{Jb}")
                    o_tiles[Jb] = oT
                else:
                    oT = o_tiles[Jb]
                l_sb = spool.tile([1, QB], F32, tag="lsb")
                nc.vector.tensor_copy(l_sb[:], o_ps[64:65, :])
                rb = spool.tile([1, QB], F32, tag="rb")
                nc.vector.reciprocal_approx_fast(rb[:], l_sb[:])
                bc = spool.tile([64, QB], F32, tag="bc")
                nc.gpsimd.partition_broadcast(bc[:], rb[:])
                nc.vector.tensor_mul(oT[r : r + 64, :], o_ps[0:64, :], bc[:])

            fillers.extend(y_pieces(Jb))

        while fillers:
            fillers.popleft()()

    nc.compile()
    return nc


# ---------------- host-side wrapper ----------------

_CACHE = {}


def _get_nc(T):
    if T not in _CACHE:
        _CACHE[T] = build_kernel(T)
    return _CACHE[T]


def _host_prep(x, cos, sin, Wq, Wk, Wv, Wp):
    T = x.shape[1]
    cosT = np.ascontiguousarray(cos.T).astype(np.float32)  # [32, T]
    sinT = np.ascontiguousarray(sin.T).astype(np.float32)
    ccT = np.concatenate([cosT] * 4, axis=0).astype(np.float16)  # [128, T]
    sgn = np.where((np.arange(128) % 64) < 32, 1.0, -1.0)[:, None].astype(np.float32)
    ssT = (np.concatenate([sinT] * 4, axis=0) * sgn).astype(np.float16)
    rr = np.arange(128)[:, None]
    cq = np.arange(512)[None, :]
    masks = np.stack(
        [(cq >= 128 * m + rr) for m in range(4)], axis=1
    ).astype(np.float16)  # [128, 4, 512]
    # qb = perm.T @ qa: qb[d] = qa[swap(d)], swap = +-32 within each 64-row head
    dd = np.arange(128)
    swap = np.where((dd % 64) < 32, dd + 32, dd - 32)
    permM = np.zeros((128, 128), np.float16)
    permM[swap, dd] = 1.0

    in_maps = []
    for core in range(N_CORES):
        b, p = core // 4, core % 4
        hs = slice(128 * p, 128 * (p + 1))
        in_maps.append(
            {
                "xT": np.ascontiguousarray(x[b].T.astype(np.float16)),
                "ccT": ccT,
                "ssT": ssT,
                "wqT": np.ascontiguousarray(Wq[hs].T).astype(np.float16),
                "wkT": np.ascontiguousarray(Wk[hs].T).astype(np.float16),
                "wvT": np.ascontiguousarray(Wv[hs].T).astype(np.float16),
                "wpT": np.ascontiguousarray(Wp[:, hs].T.astype(np.float16)),
                "masks": masks,
                "perm": permM,
            }
        )
    return in_maps


def kernel(x, cos, sin, Wq, Wk, Wv, Wp, _trace=False, _nc=None):
    x = np.asarray(x)
    T = x.shape[1]
    nc = _nc if _nc is not None else _get_nc(T)
    in_maps = _host_prep(
        x, np.asarray(cos), np.asarray(sin),
        np.asarray(Wq), np.asarray(Wk), np.asarray(Wv), np.asarray(Wp),
    )
    res = run_bass_kernel_spmd(nc, in_maps, list(range(N_CORES)), trace=_trace)
    y = np.zeros((B, T, C), np.float32)
    for core in range(N_CORES):
        y[core // 4] += res.results[core]["yT"].T.astype(np.float32)
    kernel.last_results = res
    return y
